# revision 7
# baseline (speedup 1.0000x reference)
"""Distributed Trainium2 Bass kernel for the phasor attention problem
(nn_Attention_17798344475248).

Sharding: 8 cores = 2 batches x 4 head-groups (2 heads each). Each core
computes its batch's Q/K/V projections for its 2 heads, phasor attention,
and a partial final-dense output; partials are summed with 8 pipelined
4-rank ReduceScatters (one per 128 query rows) fired as soon as the
second head completes each block; each core finishes atan2 on its
8x32-row slices of the output.

v2 structural changes vs the 933us/626us baseline:
- encodes SBUF-resident in f32r (no DRAM staging round trip); kv encodes
  scoped per-head, q encodes computed once
- q/k bias folded into ACT-square bias + scalar_tensor_tensor (no bias
  matmuls on the K/Q paths); bo folded once post-ReduceScatter
- 512-wide query chunks (half the matmul count), head-1 tail chunks of
  256 so the last ReduceScatter only covers 128 rows
- norm adds offloaded to the Pool engine; squares on ACT
- epilogue deferred to the tail (single Arctan table switch)

Numerics identical to the baseline: f32r everywhere on the data path
(bf16 was measured at 3-5e-2 rel err vs the 2e-2 gate), exp-only softmax,
normalize-instead-of-atan2+encode, fp32 PSUM accumulation.
"""
import sys

sys.path.insert(0, "/opt/trn_rl_repo")

import numpy as np

import concourse.bass as bass
import concourse.tile as tile
from concourse import bacc, mybir
from concourse.bass_utils import run_bass_kernel_spmd
from concourse.masks import make_identity

F32 = mybir.dt.float32
F32R = mybir.dt.float32r
AF = mybir.ActivationFunctionType
ALU = mybir.AluOpType
PI = float(np.pi)

B, T, D, H = 2, 1024, 512, 8
P = 128
DS = D // P          # 4 partition-slices of the model dim
ECH = 512            # encode chunk width (t)
N_CORES = 8
HPC = 2              # heads per core
RG = [[0, 1, 2, 3], [4, 5, 6, 7]]


def _norm_pair(nc, pools, re_ps, im_ps, re_out, im_out, width, bias=None):
    """Normalize complex (re+bias, im) [128,width] from PSUM to unit
    modulus. ACT: squares + sqrt; Pool: add; DVE: recip + muls.
    bias: None or a [P,1] fp32 SBUF AP (per-partition)."""
    nt = pools["nt"]
    s1 = nt.tile([P, width], F32, tag="nt")
    if bias is None:
        nc.scalar.activation(s1[:], re_ps[:], AF.Square, bias=0.0, scale=1.0)
    else:
        nc.scalar.activation(s1[:], re_ps[:], AF.Square, bias=bias, scale=1.0)
    s2 = nt.tile([P, width], F32, tag="nt")
    nc.scalar.activation(s2[:], im_ps[:], AF.Square, bias=0.0, scale=1.0)
    m = nt.tile([P, width], F32, tag="nt")
    nc.gpsimd.tensor_tensor(m[:], s1[:], s2[:], ALU.add)
    az = nt.tile([P, width], F32, tag="nt")
    nc.scalar.activation(az[:], m[:], AF.Sqrt, bias=0.0, scale=1.0)
    n = nt.tile([P, width], F32, tag="nt")
    nc.vector.reciprocal_approx_fast(n[:], az[:])
    if bias is None:
        nc.vector.tensor_tensor(re_out, re_ps[:], n[:], ALU.mult)
    else:
        nc.vector.scalar_tensor_tensor(re_out, re_ps[:], bias, n[:],
                                       ALU.add, ALU.mult)
    nc.vector.tensor_tensor(im_out, im_ps[:], n[:], ALU.mult)


def build(debug=False):
    nc = bacc.Bacc("TRN2", target_bir_lowering=False, debug=False,
                   num_devices=N_CORES)
    cpi2 = nc.alloc_sbuf_tensor("const-f32-pi2", [P, 1], F32)
    nc.gpsimd.memset(cpi2.ap(), PI / 2)
    nc.const_aps.aps[(F32, PI / 2)] = cpi2.ap()
    nc.all_engine_barrier()

    # ---- I/O ----
    QUERY = nc.dram_tensor("query", [T, D], F32, kind="ExternalInput")
    KEYVALUE = nc.dram_tensor("keyvalue", [T, D], F32, kind="ExternalInput")
    WQ = nc.dram_tensor("wq", [HPC, D, D], F32, kind="ExternalInput")
    WK = nc.dram_tensor("wk", [HPC, D, D], F32, kind="ExternalInput")
    WV = nc.dram_tensor("wv", [HPC, D, D], F32, kind="ExternalInput")
    BQ = nc.dram_tensor("bq", [HPC, D], F32, kind="ExternalInput")
    BK = nc.dram_tensor("bk", [HPC, D], F32, kind="ExternalInput")
    BV = nc.dram_tensor("bv", [HPC, D], F32, kind="ExternalInput")
    WO = nc.dram_tensor("wo", [HPC * D, D], F32, kind="ExternalInput")
    BO = nc.dram_tensor("bo", [D], F32, kind="ExternalInput")
    OUT = nc.dram_tensor("out", [T // 4, D], F32, kind="ExternalOutput")

    with tile.TileContext(nc) as tc:
        import contextlib
        with contextlib.ExitStack() as ctx:
            pools = {}
            for name, bufs, space in [
                ("persist", 1, "SBUF"), ("raw", 2, "SBUF"), ("nt", 6, "SBUF"),
                ("wr", 2, "SBUF"), ("brow", 2, "SBUF"),
                ("enc", 1, "SBUF"), ("kv", 4, "SBUF"),
                ("z", 6, "SBUF"), ("ps", 8, "PSUM"), ("dram", 1, "DRAM"),
            ]:
                pools[name] = ctx.enter_context(
                    tc.tile_pool(name=name, bufs=bufs, space=space))

            persist = pools["persist"]
            ident = persist.tile([P, P], F32, tag="ident")
            make_identity(nc, ident[:])

            # ---- constant rows / bias prep ----
            ones_row = persist.tile([1, P], F32R, tag="ones_row")
            onesf = pools["brow"].tile([1, P], F32, tag="brow", name="onesf")
            nc.vector.memset(onesf[:], 1.0)
            nc.vector.tensor_copy(ones_row[:], onesf[:])

            # bo broadcast [P, D] (added once post-RS in the epilogue)
            bo_f = pools["brow"].tile([1, D], F32, tag="brow", name="bo_f")
            nc.sync.dma_start(bo_f[:], BO[:][None, :])
            bo_r = persist.tile([1, D], F32R, tag="bor")
            nc.vector.tensor_copy(bo_r[:], bo_f[:])
            bo_ps = pools["ps"].tile([P, D], F32, tag="ps", name="bo_ps")
            nc.tensor.matmul(bo_ps[:], lhsT=ones_row[:], rhs=bo_r[:],
                             start=True, stop=True)
            bo_bc = persist.tile([P, D], F32, tag="bo_bc")
            nc.vector.tensor_copy(bo_bc[:], bo_ps[:])

            # per-head per-partition bias columns [P, DS] (col dso holds
            # b[h][dso*128:(dso+1)*128])
            bq_col = persist.tile([P, HPC, DS], F32, tag="bq_col")
            bk_col = persist.tile([P, HPC, DS], F32, tag="bk_col")
            for h in range(HPC):
                nc.sync.dma_start(
                    bq_col[:, h, :], BQ[h].rearrange("(a p) -> p a", p=P))
                nc.sync.dma_start(
                    bk_col[:, h, :], BK[h].rearrange("(a p) -> p a", p=P))
            bv_rows = []
            for h in range(HPC):
                bvf = pools["brow"].tile([1, D], F32, tag="brow",
                                         name=f"bvf{h}")
                nc.sync.dma_start(bvf[:], BV[h][None, :])
                bvr = persist.tile([1, D], F32R, tag=f"bv_row{h}")
                nc.vector.tensor_copy(bvr[:], bvf[:])
                bv_rows.append(bvr)

            # ---- DRAM staging for the reduce-scatter ----
            dram = pools["dram"]
            zbs = [dram.tile([2 * P, D], F32, name=f"zb{u}") for u in range(8)]
            rs_outs = [dram.tile([P // 2, D], F32, name=f"rsout{u}")
                       for u in range(8)]

            def encode(src_dram, cos_t, sin_t):
                """phasor-encode src [T, D] into cos/sin [P, DS, T] f32r."""
                for ch in range(T // ECH):
                    raw_tiles = []
                    for ts in range(ECH // P):
                        rt = pools["raw"].tile([P, D], F32, tag="raw")
                        nc.sync.dma_start(
                            rt[:],
                            src_dram[ch * ECH + ts * P:
                                     ch * ECH + (ts + 1) * P, :])
                        raw_tiles.append(rt)
                    chsl = slice(ch * ECH, (ch + 1) * ECH)
                    for ds in range(DS):
                        pt = pools["ps"].tile([P, ECH], F32, tag="ps")
                        for ts in range(ECH // P):
                            nc.tensor.transpose(
                                pt[:, ts * P:(ts + 1) * P],
                                raw_tiles[ts][:, ds * P:(ds + 1) * P],
                                ident[:])
                        nc.scalar.activation(sin_t[:, ds, chsl], pt[:],
                                             AF.Sin, bias=0.0, scale=PI)
                        ab = pools["nt"].tile([P, ECH], F32, tag="nt")
                        nc.scalar.activation(ab[:], pt[:], AF.Abs,
                                             bias=0.0, scale=1.0)
                        nc.scalar.activation(cos_t[:, ds, chsl], ab[:],
                                             AF.Sin, bias=PI / 2, scale=-PI)

            def load_weights_r(W_ap, name):
                w_r = pools["wr"].tile([P, DS, D], F32R, tag="wr", name=name)
                for do in range(DS):
                    wf = pools["raw"].tile([P, D], F32, tag="raw",
                                           name=f"{name}_st{do}")
                    nc.sync.dma_start(wf[:], W_ap[do * P:(do + 1) * P, :])
                    nc.vector.tensor_copy(w_r[:, do, :], wf[:])
                return w_r

            # persistent q encodes (used by both heads)
            enc = pools["enc"]
            qe_cos = enc.tile([P, DS, T], F32R, name="qe_cos")
            qe_sin = enc.tile([P, DS, T], F32R, name="qe_sin")
            encode(QUERY, qe_cos, qe_sin)

            # per-head persistent K^T / V (pool rotation reuses across heads)
            def kv_pass(h, kve_cos, kve_sin):
                wk_r = load_weights_r(WK[h], f"wk{h}")
                wv_r = load_weights_r(WV[h], f"wv{h}")
                encode(KEYVALUE, kve_cos, kve_sin)
                kt_re = pools["kv"].tile([P, DS, T], F32R, tag="kv",
                                         name=f"ktre{h}")
                kt_im = pools["kv"].tile([P, DS, T], F32R, tag="kv",
                                         name=f"ktim{h}")
                v_re = pools["kv"].tile([P, T // P, D], F32R, tag="kv",
                                        name=f"vre{h}")
                v_im = pools["kv"].tile([P, T // P, D], F32R, tag="kv",
                                        name=f"vim{h}")
                for ch in range(T // ECH):
                    chsl = slice(ch * ECH, (ch + 1) * ECH)
                    # V projection [t, D] (bias via K=1 matmul; bv general)
                    for ts in range(ECH // P):
                        tsl = slice(ch * ECH + ts * P, ch * ECH + (ts + 1) * P)
                        pre = pools["ps"].tile([P, D], F32, tag="ps")
                        pim = pools["ps"].tile([P, D], F32, tag="ps")
                        for do in range(DS):
                            nc.tensor.matmul(
                                pre[:], lhsT=kve_cos[:, do, tsl],
                                rhs=wv_r[:, do, :], start=(do == 0),
                                stop=False)
                        nc.tensor.matmul(
                            pre[:], lhsT=ones_row[:], rhs=bv_rows[h][:],
                            start=False, stop=True)
                        for do in range(DS):
                            nc.tensor.matmul(
                                pim[:], lhsT=kve_sin[:, do, tsl],
                                rhs=wv_r[:, do, :], start=(do == 0),
                                stop=(do == DS - 1))
                        trow = ch * (ECH // P) + ts
                        _norm_pair(nc, pools, pre, pim,
                                   v_re[:, trow, :], v_im[:, trow, :], D)
                    # K projection: K^T [D', t] (bias folded per-partition)
                    for dso in range(DS):
                        dsl = slice(dso * P, (dso + 1) * P)
                        pre = pools["ps"].tile([P, ECH], F32, tag="ps")
                        pim = pools["ps"].tile([P, ECH], F32, tag="ps")
                        for do in range(DS):
                            nc.tensor.matmul(
                                pre[:], lhsT=wk_r[:, do, dsl],
                                rhs=kve_cos[:, do, chsl], start=(do == 0),
                                stop=(do == DS - 1))
                        for do in range(DS):
                            nc.tensor.matmul(
                                pim[:], lhsT=wk_r[:, do, dsl],
                                rhs=kve_sin[:, do, chsl], start=(do == 0),
                                stop=(do == DS - 1))
                        _norm_pair(nc, pools, pre, pim,
                                   kt_re[:, dso, chsl], kt_im[:, dso, chsl],
                                   ECH, bias=bk_col[:, h, dso:dso + 1])
                return kt_re, kt_im, v_re, v_im

            # ================= per-head Q pass =================
            def q_pass(h, kt_re, kt_im, v_re, v_im, chunks, qtp, pp_, ohp):
                wq_r = load_weights_r(WQ[h], f"wq{h}")
                wo_r = load_weights_r(WO[h * D:(h + 1) * D, :], f"wo{h}")
                for (q0, CH) in chunks:
                    qsl = slice(q0, q0 + CH)
                    # Q^T projection [D', tq] (bias folded per-partition)
                    qt_re = qtp.tile([P, DS, CH], F32R, tag="qt",
                                     name=f"qtre{h}_{q0}")
                    qt_im = qtp.tile([P, DS, CH], F32R, tag="qt",
                                     name=f"qtim{h}_{q0}")
                    for dso in range(DS):
                        dsl = slice(dso * P, (dso + 1) * P)
                        pre = pools["ps"].tile([P, CH], F32, tag="ps")
                        pim = pools["ps"].tile([P, CH], F32, tag="ps")
                        for do in range(DS):
                            nc.tensor.matmul(
                                pre[:], lhsT=wq_r[:, do, dsl],
                                rhs=qe_cos[:, do, qsl], start=(do == 0),
                                stop=(do == DS - 1))
                        for do in range(DS):
                            nc.tensor.matmul(
                                pim[:], lhsT=wq_r[:, do, dsl],
                                rhs=qe_sin[:, do, qsl], start=(do == 0),
                                stop=(do == DS - 1))
                        _norm_pair(nc, pools, pre, pim,
                                   qt_re[:, dso, :], qt_im[:, dso, :], CH,
                                   bias=bq_col[:, h, dso:dso + 1])

                    # scores + exp -> P^T [Tkv, tq-chunk]
                    pt_all = pp_.tile([P, T // P, CH], F32R, tag="p",
                                      name=f"pt{h}_{q0}")
                    for to in range(T // P):
                        tol = slice(to * P, (to + 1) * P)
                        ps_s = pools["ps"].tile([P, CH], F32, tag="ps")
                        for do in range(DS):
                            nc.tensor.matmul(
                                ps_s[:], lhsT=kt_re[:, do, tol],
                                rhs=qt_re[:, do, :], start=(do == 0),
                                stop=False)
                        for do in range(DS):
                            nc.tensor.matmul(
                                ps_s[:], lhsT=kt_im[:, do, tol],
                                rhs=qt_im[:, do, :], start=False,
                                stop=(do == DS - 1))
                        nc.scalar.activation(pt_all[:, to, :], ps_s[:],
                                             AF.Exp, bias=0.0, scale=1.0 / D)

                    # PV: O^T [D', tq-chunk], two groups of 2 D'-slices
                    oh_re = ohp.tile([P, DS, CH], F32R, tag="oh",
                                     name=f"ohre{h}_{q0}")
                    oh_im = ohp.tile([P, DS, CH], F32R, tag="oh",
                                     name=f"ohim{h}_{q0}")
                    for grp in range(2):
                        ps_tiles = {}
                        for dso in (2 * grp, 2 * grp + 1):
                            for c in range(2):
                                ps_tiles[(dso, c)] = pools["ps"].tile(
                                    [P, CH], F32, tag="ps",
                                    name=f"pv{h}_{q0}_{dso}_{c}")
                        for to in range(T // P):
                            for dso in (2 * grp, 2 * grp + 1):
                                dsl = slice(dso * P, (dso + 1) * P)
                                nc.tensor.matmul(
                                    ps_tiles[(dso, 0)][:],
                                    lhsT=v_re[:, to, dsl],
                                    rhs=pt_all[:, to, :], start=(to == 0),
                                    stop=(to == T // P - 1))
                                nc.tensor.matmul(
                                    ps_tiles[(dso, 1)][:],
                                    lhsT=v_im[:, to, dsl],
                                    rhs=pt_all[:, to, :], start=(to == 0),
                                    stop=(to == T // P - 1))
                        for dso in (2 * grp, 2 * grp + 1):
                            _norm_pair(nc, pools, ps_tiles[(dso, 0)],
                                       ps_tiles[(dso, 1)],
                                       oh_re[:, dso, :], oh_im[:, dso, :], CH)

                    # final dense partial: Z [tq, D] += Ohat^T.T @ wo_h
                    for ts in range(CH // P):
                        tsl = slice(ts * P, (ts + 1) * P)
                        u = (q0 + ts * P) // P
                        pzre = pools["ps"].tile([P, D], F32, tag="ps")
                        pzim = pools["ps"].tile([P, D], F32, tag="ps")
                        for do in range(DS):
                            nc.tensor.matmul(
                                pzre[:], lhsT=oh_re[:, do, tsl],
                                rhs=wo_r[:, do, :], start=(do == 0),
                                stop=(do == DS - 1))
                        for do in range(DS):
                            nc.tensor.matmul(
                                pzim[:], lhsT=oh_im[:, do, tsl],
                                rhs=wo_r[:, do, :], start=(do == 0),
                                stop=(do == DS - 1))
                        # zbs[u] rank-packed view: [4 ranks, re/im, 32, D]
                        zb_v = zbs[u][:, :].rearrange(
                            "(r c j) D -> r c j D", r=4, c=2)
                        if h == 0:
                            zre_sb = pools["z"].tile([P, D], F32, tag="z")
                            zim_sb = pools["z"].tile([P, D], F32, tag="z")
                            nc.vector.tensor_copy(zre_sb[:], pzre[:])
                            nc.vector.tensor_copy(zim_sb[:], pzim[:])
                            for r in range(4):
                                rsl = slice(r * 32, (r + 1) * 32)
                                nc.sync.dma_start(zb_v[r, 0, :, :],
                                                  zre_sb[rsl, :])
                                nc.sync.dma_start(zb_v[r, 1, :, :],
                                                  zim_sb[rsl, :])
                        else:
                            h0re = pools["z"].tile([P, D], F32, tag="z",
                                                   name=f"h0re_{u}")
                            h0im = pools["z"].tile([P, D], F32, tag="z",
                                                   name=f"h0im_{u}")
                            for r in range(4):
                                rsl = slice(r * 32, (r + 1) * 32)
                                nc.sync.dma_start(h0re[rsl, :],
                                                  zb_v[r, 0, :, :])
                                nc.sync.dma_start(h0im[rsl, :],
                                                  zb_v[r, 1, :, :])
                            zre_sb = pools["z"].tile([P, D], F32, tag="z")
                            zim_sb = pools["z"].tile([P, D], F32, tag="z")
                            nc.vector.tensor_tensor(zre_sb[:], pzre[:],
                                                    h0re[:], ALU.add)
                            nc.vector.tensor_tensor(zim_sb[:], pzim[:],
                                                    h0im[:], ALU.add)
                            for r in range(4):
                                rsl = slice(r * 32, (r + 1) * 32)
                                nc.sync.dma_start(zb_v[r, 0, :, :],
                                                  zre_sb[rsl, :])
                                nc.sync.dma_start(zb_v[r, 1, :, :],
                                                  zim_sb[rsl, :])
                            nc.gpsimd.collective_compute(
                                "ReduceScatter", ALU.add,
                                replica_groups=RG,
                                ins=[zbs[u].opt()],
                                outs=[rs_outs[u].opt()],
                            )

            for h in range(HPC):
                with tc.tile_pool(name=f"kve{h}", bufs=1) as kvep:
                    kve_cos = kvep.tile([P, DS, T], F32R, name=f"kvec{h}")
                    kve_sin = kvep.tile([P, DS, T], F32R, name=f"kves{h}")
                    kt = kv_pass(h, kve_cos, kve_sin)
                with tc.tile_pool(name=f"qt{h}", bufs=2) as qtp, \
                        tc.tile_pool(name=f"p{h}", bufs=1) as pp_, \
                        tc.tile_pool(name=f"oh{h}", bufs=2) as ohp:
                    chunks = ([(0, 512), (512, 512)] if h == 0 else
                              [(0, 512), (512, 256), (768, 256)])
                    q_pass(h, *kt, chunks=chunks, qtp=qtp, pp_=pp_, ohp=ohp)

            # ======== epilogue: atan2(zim, zre + bo)/pi ========
            # OUT row u*32+j <-> global tq row u*128 + g*32 + j (g = rank)
            for pp in range(2):
                zre_t = pools["z"].tile([P, D], F32, tag="z", name=f"zre{pp}")
                zim_t = pools["z"].tile([P, D], F32, tag="z", name=f"zim{pp}")
                for k in range(4):
                    u = pp * 4 + k
                    ksl = slice(k * 32, (k + 1) * 32)
                    nc.sync.dma_start(zre_t[ksl, :], rs_outs[u][0:32, :])
                    nc.sync.dma_start(zim_t[ksl, :], rs_outs[u][32:64, :])
                nt = pools["nt"]

                def ft(nm, pp=pp):
                    return nt.tile([P, D], F32, tag="nt", name=f"{nm}{pp}")
                zre = ft("zrb")
                nc.vector.tensor_tensor(zre[:], zre_t[:], bo_bc[:], ALU.add)
                zim = zim_t[:, :]
                t1 = ft("f1")
                nc.scalar.activation(t1[:], zre[:], AF.Square, bias=0.0,
                                     scale=1.0)
                t2 = ft("f2")
                nc.vector.tensor_tensor(t2[:], zim, zim, ALU.mult)
                m = ft("f3")
                nc.gpsimd.tensor_tensor(m[:], t1[:], t2[:], ALU.add)
                az = ft("f5")
                nc.scalar.activation(az[:], m[:], AF.Sqrt, bias=0.0, scale=1.0)
                den1 = ft("f6")
                nc.vector.tensor_tensor(den1[:], az[:], zre[:], ALU.add)
                r1 = ft("f7")
                nc.vector.reciprocal_approx_fast(r1[:], den1[:])
                ta0 = ft("f8")
                nc.vector.tensor_tensor(ta0[:], zim, r1[:], ALU.mult)
                ta = ft("f9")
                nc.vector.tensor_scalar(ta[:], ta0[:], 1e8, -1e8, ALU.min,
                                        ALU.max)
                num2 = ft("fa")
                nc.vector.tensor_tensor(num2[:], az[:], zre[:], ALU.subtract)
                r2 = ft("fb")
                nc.vector.reciprocal_approx_fast(r2[:], zim)
                tb0 = ft("fc")
                nc.vector.tensor_tensor(tb0[:], num2[:], r2[:], ALU.mult)
                tb = ft("fd")
                nc.vector.tensor_scalar(tb[:], tb0[:], 1e8, -1e8, ALU.min,
                                        ALU.max)
                ata = ft("fe")
                nc.scalar.activation(ata[:], ta[:], AF.Arctan, bias=0.0,
                                     scale=1.0)
                atb = ft("ff")
                nc.scalar.activation(atb[:], tb[:], AF.Arctan, bias=0.0,
                                     scale=1.0)
                mask = ft("fg")
                nc.vector.tensor_scalar(mask[:], zre[:], 0.0, None, ALU.is_ge)
                dsel = ft("fh")
                nc.vector.tensor_tensor(dsel[:], ata[:], atb[:], ALU.subtract)
                md = ft("fi")
                nc.vector.tensor_tensor(md[:], mask[:], dsel[:], ALU.mult)
                sel = ft("fj")
                nc.vector.tensor_tensor(sel[:], atb[:], md[:], ALU.add)
                outt = ft("fk")
                nc.vector.tensor_scalar(outt[:], sel[:], 2.0 / PI, None,
                                        ALU.mult)
                nc.sync.dma_start(OUT[pp * P:(pp + 1) * P, :], outt[:])

    nc.finalize()
    return nc


_NC_CACHE = {}


def _get_nc():
    if "nc" not in _NC_CACHE:
        _NC_CACHE["nc"] = build()
    return _NC_CACHE["nc"]


def kernel(**inputs):
    query = np.ascontiguousarray(np.asarray(inputs["query"], dtype=np.float32))
    keyvalue = np.ascontiguousarray(
        np.asarray(inputs["keyvalue"], dtype=np.float32))
    wq = np.asarray(inputs["wq"], dtype=np.float32)
    wk = np.asarray(inputs["wk"], dtype=np.float32)
    wv = np.asarray(inputs["wv"], dtype=np.float32)
    bq = np.asarray(inputs["bq"], dtype=np.float32)
    bk = np.asarray(inputs["bk"], dtype=np.float32)
    bv = np.asarray(inputs["bv"], dtype=np.float32)
    wo = np.asarray(inputs["wo"], dtype=np.float32)
    bo = np.asarray(inputs["bo"], dtype=np.float32)

    in_maps = []
    for c in range(N_CORES):
        b, g = c // 4, c % 4
        h0 = g * HPC
        in_maps.append({
            "query": query[b],
            "keyvalue": keyvalue[b],
            "wq": np.ascontiguousarray(wq[h0:h0 + HPC]),
            "wk": np.ascontiguousarray(wk[h0:h0 + HPC]),
            "wv": np.ascontiguousarray(wv[h0:h0 + HPC]),
            "bq": np.ascontiguousarray(bq[h0:h0 + HPC]),
            "bk": np.ascontiguousarray(bk[h0:h0 + HPC]),
            "bv": np.ascontiguousarray(bv[h0:h0 + HPC]),
            "wo": np.ascontiguousarray(wo[h0 * D:(h0 + HPC) * D]),
            "bo": bo,
        })

    nc = _get_nc()
    res = run_bass_kernel_spmd(nc, in_maps, core_ids=list(range(N_CORES)))
    _NC_CACHE["last_results"] = res
    out = np.empty((B, T, D), np.float32)
    for c in range(N_CORES):
        b, g = c // 4, c % 4
        o = res.results[c]["out"]          # [256, 512]: 8 u-blocks x 32 rows
        for u in range(8):
            out[b, u * P + g * 32: u * P + (g + 1) * 32, :] = \
                o[u * 32:(u + 1) * 32, :]
    return out
